# revision 1
# baseline (speedup 1.0000x reference)
import numpy as np
import jax
import jax.numpy as jnp

# Hardcoded problem shapes (nn_Attention_41532333753073)
B, T, DM = 2, 2048, 2048
H, DH = 32, 64
ROPE_THETA = 10000.0
N_CORES = 8
GROUPS = 4           # head-groups per batch (tensor-parallel)
HG = H // GROUPS     # heads per group = 8
DG = HG * DH         # 512 cols per group


def _rope_cos_sin(seq_len, dim, theta):
    inv_freq = 1.0 / (theta ** (np.arange(0, dim, 2, dtype=np.float32) / dim))
    t = np.arange(seq_len, dtype=np.float32)
    freqs = np.outer(t, inv_freq)
    return np.cos(freqs).astype(np.float32), np.sin(freqs).astype(np.float32)


_COS_NP, _SIN_NP = _rope_cos_sin(T, DH, ROPE_THETA)


def _shard_math(hidden, v1s, Wq_g, Wk_g, Wv_g, Wo_g, lambda1, lambda2, cos, sin):
    # hidden: [T, DM]; v1s: [T, HG, DH]; W*_g: [DM, DG]; Wo_g: [DG, DM]
    q = (hidden @ Wq_g).reshape(T, HG, DH)
    k = (hidden @ Wk_g).reshape(T, HG, DH)
    v = (hidden @ Wv_g).reshape(T, HG, DH)
    v = lambda1 * v1s + lambda2 * v

    d2 = DH // 2
    c = cos[:, None, :]
    s = sin[:, None, :]

    def rope(x):
        x1, x2 = x[..., :d2], x[..., d2:]
        return jnp.concatenate([x1 * c - x2 * s, x2 * c + x1 * s], axis=-1)

    q = rope(q)
    k = rope(k)
    scale = 1.0 / np.sqrt(DH)
    scores = jnp.einsum("qhd,khd->hqk", q, k) * scale
    causal = jnp.tril(jnp.ones((T, T), dtype=bool))
    scores = jnp.where(causal[None], scores, jnp.finfo(scores.dtype).min)
    probs = jax.nn.softmax(scores, axis=-1)
    o = jnp.einsum("hqk,khd->qhd", probs, v).reshape(T, DG)
    return o @ Wo_g  # [T, DM] partial (sum over 4 groups gives full output)


def _run_sharded(hidden_states, v1, lambda1, Wq, Wk, Wv, Wo, lambda2):
    # core i -> batch i//GROUPS, head-group i%GROUPS
    hid_sh = np.empty((N_CORES, T, DM), np.float32)
    v1_sh = np.empty((N_CORES, T, HG, DH), np.float32)
    wq_sh = np.empty((N_CORES, DM, DG), np.float32)
    wk_sh = np.empty((N_CORES, DM, DG), np.float32)
    wv_sh = np.empty((N_CORES, DM, DG), np.float32)
    wo_sh = np.empty((N_CORES, DG, DM), np.float32)
    for i in range(N_CORES):
        b, g = divmod(i, GROUPS)
        cs = slice(g * DG, (g + 1) * DG)
        hs = slice(g * HG, (g + 1) * HG)
        hid_sh[i] = hidden_states[b]
        v1_sh[i] = v1[b, :, hs, :]
        wq_sh[i] = Wq[:, cs]
        wk_sh[i] = Wk[:, cs]
        wv_sh[i] = Wv[:, cs]
        wo_sh[i] = Wo[cs, :]
    lam1 = np.full((N_CORES,), np.float32(lambda1))
    lam2 = np.full((N_CORES,), np.float32(lambda2))
    cos_sh = np.broadcast_to(_COS_NP, (N_CORES,) + _COS_NP.shape)
    sin_sh = np.broadcast_to(_SIN_NP, (N_CORES,) + _SIN_NP.shape)

    fn = jax.pmap(_shard_math, devices=jax.devices()[:N_CORES])
    parts = fn(hid_sh, v1_sh, wq_sh, wk_sh, wv_sh, wo_sh, lam1, lam2,
               cos_sh, sin_sh)
    parts = np.asarray(parts)  # [8, T, DM]
    out = np.empty((B, T, DM), np.float32)
    for b in range(B):
        out[b] = parts[b * GROUPS:(b + 1) * GROUPS].sum(axis=0)
    return out


def _run_host(hidden_states, v1, lambda1, Wq, Wk, Wv, Wo, lambda2):
    # CPU fallback: exact reference math, single device.
    cpu = jax.devices("cpu")[0]
    with jax.default_device(cpu):
        q = (hidden_states @ Wq).reshape(B, T, H, DH)
        k = (hidden_states @ Wk).reshape(B, T, H, DH)
        v = (hidden_states @ Wv).reshape(B, T, H, DH)
        v = lambda1 * v1 + lambda2 * v
        c = jnp.asarray(_COS_NP)[None, :, None, :]
        s = jnp.asarray(_SIN_NP)[None, :, None, :]
        d2 = DH // 2

        def rope(x):
            x1, x2 = x[..., :d2], x[..., d2:]
            return jnp.concatenate([x1 * c - x2 * s, x2 * c + x1 * s], axis=-1)

        q = rope(jnp.asarray(q))
        k = rope(jnp.asarray(k))
        scale = 1.0 / np.sqrt(DH)
        scores = jnp.einsum("bqhd,bkhd->bhqk", q, k) * scale
        causal = jnp.tril(jnp.ones((T, T), dtype=bool))
        scores = jnp.where(causal[None, None], scores,
                           jnp.finfo(scores.dtype).min)
        probs = jax.nn.softmax(scores, axis=-1)
        o = jnp.einsum("bhqk,bkhd->bqhd", probs, jnp.asarray(v))
        o = o.reshape(B, T, DM)
        return np.asarray(o @ Wo, dtype=np.float32)


def kernel(hidden_states, v1, lambda1, Wq, Wk, Wv, Wo, lambda2):
    args = (np.asarray(hidden_states, np.float32), np.asarray(v1, np.float32),
            np.float32(lambda1), np.asarray(Wq, np.float32),
            np.asarray(Wk, np.float32), np.asarray(Wv, np.float32),
            np.asarray(Wo, np.float32), np.float32(lambda2))
    try:
        if len(jax.devices()) >= N_CORES:
            return _run_sharded(*args)
    except Exception:
        pass
    return _run_host(*args)
